# revision 9
# baseline (speedup 1.0000x reference)
"""Trainium2 Bass kernel for nn_AffinityHead (GNN edge-MLP affinity head).

Math: out[e] = w2 . relu(W1a.x_src + W1b.x_dst + W1c.(c_dst - c_src) + b1) + b2

Decomposition into per-node tables (built on device, phase Z):
    z1[n] = x_n@W1a - c_n@W1c
    z2[n] = x_n@W1b + c_n@W1c + b1      (b1 folded in via a ones-row matmul)
so per edge (phase G):  out[e] = w2 . relu(z1[src] + z2[dst]) + b2.

w2 is folded into the tables as |w2| scaling + sign-ordered channel permutation
(host-side weight prep), so the per-edge tail is relu + two free-range reduces.

Device (8 cores SPMD, edges sharded E/8 per core):
  Phase Z: PE matmuls (lhsT = host-transposed token tile) build z rows,
  written interleaved [z1[n] | z2[n]] per node to an HBM table [N, 256].
  Phase G: per 1024-edge tile, two SWDGE dma_gather calls fetch z1[src]/z2[dst]
  rows; gathers round-robin over all 4 SWDGE queues (each queue's descriptor
  generation runs on a different Q7 core pair — measured 2.8x throughput vs one
  queue). DVE adds, ACT relu, DVE range-reduces -> per-edge scalar.

dma_gather's index operand is int16, so node ids >= 32768 are handled by
bucketing edges (host) by (src>=32768, dst>=32768) and slicing the table.
"""

import numpy as np
import ml_dtypes

# Problem constants (hardcoded per harness contract)
N, C, E = 50000, 128, 800000
N_PAD = 50176  # 2048-multiple node padding for phase-Z macro tiles
N_CORES = 8
EC = E // N_CORES
HALF = 32768
TBL_BF16 = True         # table dtype knob
TILE_T = 1024            # edges per gather tile (ring-entry limit: see notes)
GP = TILE_T // 128
MT = 2048                # phase-Z tokT macro tile columns
NQ = 4                   # SWDGE queues

_cache = {}


def _build(caps, p_pos, tile_bounds):
    import concourse.bacc as bacc
    import concourse.mybir as mybir
    import concourse.tile as tile

    tb_dt = mybir.dt.bfloat16 if TBL_BF16 else mybir.dt.float32
    CAPT = sum(caps)
    IW = CAPT // 16
    NTT = CAPT // TILE_T

    nc = bacc.Bacc("TRN2", target_bir_lowering=False, debug=False,
                   num_devices=N_CORES, num_swdge_queues=NQ)

    tokT = nc.dram_tensor("tokT", [C, N_PAD], tb_dt, kind="ExternalInput").ap()
    cooT = nc.dram_tensor("cooT", [4, N_PAD], tb_dt, kind="ExternalInput").ap()
    w1ab = nc.dram_tensor("w1ab", [C, 2 * C], tb_dt, kind="ExternalInput").ap()
    w1c3 = nc.dram_tensor("w1c3", [4, 2 * C], tb_dt, kind="ExternalInput").ap()
    sidx = nc.dram_tensor("sidx", [128, IW], mybir.dt.int16, kind="ExternalInput").ap()
    didx = nc.dram_tensor("didx", [128, IW], mybir.dt.int16, kind="ExternalInput").ap()
    outd = nc.dram_tensor("out", [128, NTT * GP], mybir.dt.float32,
                          kind="ExternalOutput").ap()
    z_dram = nc.dram_tensor("ztbl", [N_PAD, 2 * C], tb_dt).ap()

    with tile.TileContext(nc) as tc:
        with (
            tc.tile_pool(name="wpool", bufs=1) as wpool,
            tc.tile_pool(name="zpsum", bufs=6, space="PSUM") as zpsum,
            tc.tile_pool(name="ztok", bufs=2) as ztok,
            tc.tile_pool(name="zcoo", bufs=2) as zcoo,
            tc.tile_pool(name="zstage", bufs=2) as zstage,
            tc.tile_pool(name="ipool", bufs=1) as ipool,
            tc.tile_pool(name="gpool", bufs=8) as gpool,
            tc.tile_pool(name="spool", bufs=3) as spool,
            tc.tile_pool(name="rpool", bufs=3) as rpool,
            tc.tile_pool(name="opool", bufs=3) as opool,
            tc.tile_pool(name="obuf", bufs=1) as obuf,
        ):
            # edge-index tiles load first so gathers are never queued behind
            # the phase-Z DMA stream
            sidx_sb = ipool.tile([128, IW], mybir.dt.int16)
            nc.sync.dma_start(out=sidx_sb[:], in_=sidx[:])
            didx_sb = ipool.tile([128, IW], mybir.dt.int16)
            nc.sync.dma_start(out=didx_sb[:], in_=didx[:])
            outbuf = obuf.tile([128, NTT * GP], mybir.dt.float32)

            # ---------------- Phase Z: build z tables ----------------
            w1ab_sb = wpool.tile([C, 2 * C], tb_dt)
            nc.sync.dma_start(out=w1ab_sb[:], in_=w1ab[:])
            w1c3_sb = wpool.tile([4, 2 * C], tb_dt)
            nc.sync.dma_start(out=w1c3_sb[:], in_=w1c3[:])

            NG = MT // 128
            zwrites = []
            for m in range(0, N_PAD, MT):
                mw = min(MT, N_PAD - m)
                ng = mw // 128
                tok_mt = ztok.tile([C, MT], tb_dt, tag="tok")
                nc.sync.dma_start(out=tok_mt[:, :mw], in_=tokT[:, m:m + mw])
                coo_mt = zcoo.tile([4, MT], tb_dt, tag="coo")
                nc.sync.dma_start(out=coo_mt[:, :mw], in_=cooT[:, m:m + mw])
                zs = zstage.tile([128, NG, 2 * C], tb_dt, tag="zs")
                for g in range(ng):
                    cc = g * 128
                    ps = zpsum.tile([128, 2 * C], mybir.dt.float32, tag="ps")
                    nc.tensor.matmul(ps[:], lhsT=tok_mt[:, cc:cc + 128],
                                     rhs=w1ab_sb[:], start=True, stop=False)
                    nc.tensor.matmul(ps[:], lhsT=coo_mt[:, cc:cc + 128],
                                     rhs=w1c3_sb[:], start=False, stop=True)
                    # split the PSUM drain across ACT and DVE
                    nc.scalar.copy(out=zs[:, g, 0:C], in_=ps[:, 0:C])
                    nc.vector.tensor_copy(out=zs[:, g, C:2 * C], in_=ps[:, C:2 * C])
                # one batched table write per macro tile: row m+g*128+p <- zs[p, g, :]
                zv = z_dram[m:m + mw, :].rearrange("(g p) c -> p g c", p=128)
                zw = nc.sync.dma_start(out=zv, in_=zs[:, :ng, :])
                zwrites.append(zw.ins)

            # ---------------- Phase G: gather + edge tail ----------------
            # gathers are ordered against z-table writes via explicit per-tile
            # deps (tile_bounds) + sliced in_aps, so phase G overlaps phase Z
            from concourse.tile_rust import add_dep_helper

            tt = 0
            off = 0
            qn = 0
            for b in range(4):
                if caps[b] == 0:
                    continue
                src_hi, dst_hi = b >= 2, (b % 2) == 1
                lo1 = HALF if src_hi else 0
                lo2 = HALF if dst_hi else 0
                for t in range(caps[b] // TILE_T):
                    col0 = (off + t * TILE_T) // 16
                    cols = TILE_T // 16
                    b1g, b2g = tile_bounds[tt]
                    # slice the source to just the rows this tile touches, so
                    # Tile's DRAM RAW tracking only orders against the z-writes
                    # that actually cover them (lets gathers overlap phase Z)
                    hi1t = min(N_PAD, -(-(b1g + 1) // MT) * MT)
                    hi2t = min(N_PAD, -(-(b2g + 1) // MT) * MT)
                    z1_ap = z_dram[lo1:hi1t, 0:C]
                    z2_ap = z_dram[lo2:hi2t, C:2 * C]
                    g1 = gpool.tile([128, GP, C], tb_dt, tag="g1")
                    gi1 = nc.gpsimd.dma_gather(
                        out_ap=g1[:], in_ap=z1_ap,
                        idxs_ap=sidx_sb[:, col0:col0 + cols],
                        num_idxs=TILE_T, num_idxs_reg=TILE_T,
                        elem_size=C, elem_step=2 * C, queue_num=qn % NQ)
                    add_dep_helper(gi1.ins, zwrites[b1g // MT],
                                   reason="z rows ready for g1")
                    qn += 1
                    g2 = gpool.tile([128, GP, C], tb_dt, tag="g2")
                    gi2 = nc.gpsimd.dma_gather(
                        out_ap=g2[:], in_ap=z2_ap,
                        idxs_ap=didx_sb[:, col0:col0 + cols],
                        num_idxs=TILE_T, num_idxs_reg=TILE_T,
                        elem_size=C, elem_step=2 * C, queue_num=qn % NQ)
                    add_dep_helper(gi2.ins, zwrites[b2g // MT],
                                   reason="z rows ready for g2")
                    qn += 1
                    s = spool.tile([128, GP, C], tb_dt, tag="s")
                    nc.vector.tensor_add(s[:], g1[:], g2[:])
                    r = rpool.tile([128, GP, C], tb_dt, tag="r")
                    nc.scalar.activation(r[:], s[:],
                                         mybir.ActivationFunctionType.Relu)
                    o_pos = opool.tile([128, GP], mybir.dt.float32, tag="op")
                    o_neg = opool.tile([128, GP], mybir.dt.float32, tag="on")
                    if p_pos > 0:
                        nc.vector.reduce_sum(o_pos[:], r[:, :, 0:p_pos],
                                             axis=mybir.AxisListType.X)
                    else:
                        nc.vector.memset(o_pos[:], 0.0)
                    if p_pos < C:
                        nc.vector.reduce_sum(o_neg[:], r[:, :, p_pos:C],
                                             axis=mybir.AxisListType.X)
                    else:
                        nc.vector.memset(o_neg[:], 0.0)
                    nc.vector.tensor_sub(
                        outbuf[:, tt * GP:(tt + 1) * GP], o_pos[:], o_neg[:])
                    tt += 1
                off += caps[b]
            nc.sync.dma_start(out=outd[:], in_=outbuf[:])

    nc.compile()
    return nc


def _prep_host(tokens, coords, edge_index, w1, b1, w2, b2):
    tokens = np.asarray(tokens, dtype=np.float32)[0]          # [N, C]
    coords = np.asarray(coords, dtype=np.float32)[0]          # [N, 2]
    ei = np.asarray(edge_index).astype(np.int64)              # [2, E]
    w1 = np.asarray(w1, dtype=np.float32)
    b1 = np.asarray(b1, dtype=np.float32)
    w2 = np.asarray(w2, dtype=np.float32)
    b2 = np.asarray(b2, dtype=np.float32)

    w2v = w2[:, 0]
    order = np.argsort(w2v < 0, kind="stable")
    p_pos = int((w2v >= 0).sum())
    scale = np.abs(w2v[order])
    w1p = w1[:, order] * scale[None, :]
    b1p = b1[order] * scale

    W1a, W1b = w1p[:C], w1p[C:2 * C]
    W1cx, W1cy = w1p[2 * C], w1p[2 * C + 1]

    np_tb = ml_dtypes.bfloat16 if TBL_BF16 else np.float32
    tokT_np = np.zeros((C, N_PAD), dtype=np.float32)
    tokT_np[:, :N] = tokens.T
    tokT_np = tokT_np.astype(np_tb)
    cooT_np = np.zeros((4, N_PAD), dtype=np.float32)
    cooT_np[0, :N] = coords[:, 0]
    cooT_np[1, :N] = coords[:, 1]
    cooT_np[2, :] = 1.0
    w1ab_np = np.concatenate([W1a, W1b], axis=1).astype(np_tb)  # [C, 256]
    w1c3_np = np.zeros((4, 2 * C), dtype=np.float32)
    w1c3_np[0] = np.concatenate([-W1cx, W1cx])
    w1c3_np[1] = np.concatenate([-W1cy, W1cy])
    w1c3_np[2] = np.concatenate([np.zeros(C, np.float32), b1p])
    cooT_np = cooT_np.astype(np_tb)
    w1c3_np = w1c3_np.astype(np_tb)

    src, dst = ei[0], ei[1]
    per_core = []
    counts = np.zeros((N_CORES, 4), dtype=np.int64)
    for c in range(N_CORES):
        s_ = src[c * EC:(c + 1) * EC]
        d_ = dst[c * EC:(c + 1) * EC]
        bid = (s_ >= HALF).astype(np.int64) * 2 + (d_ >= HALF).astype(np.int64)
        bound = np.maximum(s_, d_)
        pos = []
        for k in range(4):
            pk = np.nonzero(bid == k)[0]
            pos.append(pk[np.argsort(bound[pk], kind="stable")])
        for k in range(4):
            counts[c, k] = len(pos[k])
        per_core.append((s_, d_, pos))

    caps = tuple(int(-(-counts[:, k].max() // TILE_T) * TILE_T) for k in range(4))
    CAPT = sum(caps)

    in_maps = []
    pos_maps = []
    # per-tile max z-row needed, maximized across cores (SPMD: one program)
    NTT = CAPT // TILE_T
    tb1 = np.zeros(NTT, dtype=np.int64)
    tb2 = np.zeros(NTT, dtype=np.int64)
    for c in range(N_CORES):
        s_, d_, pos = per_core[c]
        sl = np.zeros(CAPT, dtype=np.int16)
        dl = np.zeros(CAPT, dtype=np.int16)
        pm = np.full(CAPT, -1, dtype=np.int64)
        off = 0
        for k in range(4):
            p = pos[k]
            n = len(p)
            sl[off:off + n] = s_[p] - (HALF if k >= 2 else 0)
            dl[off:off + n] = d_[p] - (HALF if (k % 2) == 1 else 0)
            pm[off:off + n] = p
            off += caps[k]
        base1 = np.zeros(CAPT, dtype=np.int64)
        base2 = np.zeros(CAPT, dtype=np.int64)
        o2 = 0
        for k in range(4):
            base1[o2:o2 + caps[k]] = HALF if k >= 2 else 0
            base2[o2:o2 + caps[k]] = HALF if (k % 2) == 1 else 0
            o2 += caps[k]
        r1 = sl.astype(np.int64) + base1
        r2 = dl.astype(np.int64) + base2
        np.maximum(tb1, r1.reshape(NTT, TILE_T).max(1), out=tb1)
        np.maximum(tb2, r2.reshape(NTT, TILE_T).max(1), out=tb2)
        sw = np.tile(np.ascontiguousarray(sl.reshape(-1, 16).T), (8, 1))
        dw = np.tile(np.ascontiguousarray(dl.reshape(-1, 16).T), (8, 1))
        in_maps.append({
            "tokT": tokT_np, "cooT": cooT_np, "w1ab": w1ab_np, "w1c3": w1c3_np,
            "sidx": np.ascontiguousarray(sw), "didx": np.ascontiguousarray(dw),
        })
        pos_maps.append(pm)
    tile_bounds = tuple(zip(tb1.tolist(), tb2.tolist()))
    return caps, p_pos, in_maps, pos_maps, float(b2[0]), tile_bounds


def kernel(tokens, coords, edge_index, w1, b1, w2, b2):
    from concourse.bass_utils import run_bass_kernel_spmd

    caps, p_pos, in_maps, pos_maps, b2v, tile_bounds = _prep_host(
        tokens, coords, edge_index, w1, b1, w2, b2)

    key = (caps, p_pos, tile_bounds)
    if key not in _cache:
        _cache[key] = _build(caps, p_pos, tile_bounds)
    nc = _cache[key]

    last_err = None
    for _attempt in range(3):
        try:
            res = run_bass_kernel_spmd(nc, in_maps, list(range(N_CORES)))
            break
        except Exception as e:  # transient NRT exec-unit errors observed
            last_err = e
            import time as _time
            _time.sleep(20)
    else:
        raise last_err

    CAPT = sum(caps)
    NTT = CAPT // TILE_T
    out = np.empty(E, dtype=np.float32)
    for c in range(N_CORES):
        o = res.results[c]["out"]                   # [128, NTT*GP]
        # element [p, tt*GP + g] is bucket-position tt*TILE_T + g*128 + p
        r = o.reshape(128, NTT, GP).transpose(1, 2, 0).reshape(-1)
        pm = pos_maps[c]
        valid = pm >= 0
        out[c * EC + pm[valid]] = r[valid]
    out += b2v
    return out.reshape(1, E, 1)


# revision 10
# speedup vs baseline: 1.0641x; 1.0641x over previous
"""Trainium2 Bass kernel for nn_AffinityHead (GNN edge-MLP affinity head).

Math: out[e] = w2 . relu(W1a.x_src + W1b.x_dst + W1c.(c_dst - c_src) + b1) + b2

Decomposition into per-node tables (built on device, phase Z):
    z1[n] = x_n@W1a - c_n@W1c
    z2[n] = x_n@W1b + c_n@W1c + b1      (b1 folded in via a ones-row matmul)
so per edge (phase G):  out[e] = w2 . relu(z1[src] + z2[dst]) + b2.

w2 is folded into the tables as |w2| scaling + sign-ordered channel permutation
(host-side weight prep), so the per-edge tail is relu + two free-range reduces.

Device (8 cores SPMD, edges sharded E/8 per core):
  Phase Z: PE matmuls (lhsT = host-transposed token tile) build z rows,
  written interleaved [z1[n] | z2[n]] per node to an HBM table [N, 256].
  Phase G: per 1024-edge tile, two SWDGE dma_gather calls fetch z1[src]/z2[dst]
  rows; gathers round-robin over all 4 SWDGE queues (each queue's descriptor
  generation runs on a different Q7 core pair — measured 2.8x throughput vs one
  queue). DVE adds, ACT relu, DVE range-reduces -> per-edge scalar.

dma_gather's index operand is int16, so node ids >= 32768 are handled by
bucketing edges (host) by (src>=32768, dst>=32768) and slicing the table.
"""

import numpy as np
import ml_dtypes

# Problem constants (hardcoded per harness contract)
N, C, E = 50000, 128, 800000
N_PAD = 50176  # 2048-multiple node padding for phase-Z macro tiles
N_CORES = 8
EC = E // N_CORES
HALF = 32768
TBL_BF16 = True         # table dtype knob
TILE_T = 1024            # edges per gather tile (ring-entry limit: see notes)
GP = TILE_T // 128
MT = 2048                # phase-Z tokT macro tile columns
NQ = 4                   # SWDGE queues

_cache = {}


def _build(caps, p_pos, tile_bounds):
    import concourse.bacc as bacc
    import concourse.mybir as mybir
    import concourse.tile as tile

    tb_dt = mybir.dt.bfloat16 if TBL_BF16 else mybir.dt.float32
    CAPT = sum(caps)
    IW = CAPT // 16
    NTT = CAPT // TILE_T

    nc = bacc.Bacc("TRN2", target_bir_lowering=False, debug=False,
                   num_devices=N_CORES, num_swdge_queues=NQ)

    tokT = nc.dram_tensor("tokT", [C, N_PAD], tb_dt, kind="ExternalInput").ap()
    cooT = nc.dram_tensor("cooT", [4, N_PAD], tb_dt, kind="ExternalInput").ap()
    w1ab = nc.dram_tensor("w1ab", [C, 2 * C], tb_dt, kind="ExternalInput").ap()
    w1c3 = nc.dram_tensor("w1c3", [4, 2 * C], tb_dt, kind="ExternalInput").ap()
    sidx = nc.dram_tensor("sidx", [128, IW], mybir.dt.int16, kind="ExternalInput").ap()
    didx = nc.dram_tensor("didx", [128, IW], mybir.dt.int16, kind="ExternalInput").ap()
    outd = nc.dram_tensor("out", [128, NTT * GP], mybir.dt.float32,
                          kind="ExternalOutput").ap()
    z_dram = nc.dram_tensor("ztbl", [N_PAD, 2 * C], tb_dt).ap()

    with tile.TileContext(nc) as tc:
        with (
            tc.tile_pool(name="wpool", bufs=1) as wpool,
            tc.tile_pool(name="zpsum", bufs=6, space="PSUM") as zpsum,
            tc.tile_pool(name="ztok", bufs=2) as ztok,
            tc.tile_pool(name="zcoo", bufs=2) as zcoo,
            tc.tile_pool(name="zstage", bufs=2) as zstage,
            tc.tile_pool(name="ipool", bufs=1) as ipool,
            tc.tile_pool(name="gpool", bufs=12) as gpool,
            tc.tile_pool(name="spool", bufs=4) as spool,
            tc.tile_pool(name="rpool", bufs=4) as rpool,
            tc.tile_pool(name="opool", bufs=4) as opool,
            tc.tile_pool(name="obuf", bufs=1) as obuf,
        ):
            # edge-index tiles load first so gathers are never queued behind
            # the phase-Z DMA stream
            sidx_sb = ipool.tile([128, IW], mybir.dt.int16)
            nc.sync.dma_start(out=sidx_sb[:], in_=sidx[:])
            didx_sb = ipool.tile([128, IW], mybir.dt.int16)
            nc.sync.dma_start(out=didx_sb[:], in_=didx[:])
            outbuf = obuf.tile([128, NTT * GP], mybir.dt.float32)

            # ---------------- Phase Z: build z tables ----------------
            w1ab_sb = wpool.tile([C, 2 * C], tb_dt)
            nc.sync.dma_start(out=w1ab_sb[:], in_=w1ab[:])
            w1c3_sb = wpool.tile([4, 2 * C], tb_dt)
            nc.sync.dma_start(out=w1c3_sb[:], in_=w1c3[:])

            NG = MT // 128
            zwrites = []
            for m in range(0, N_PAD, MT):
                mw = min(MT, N_PAD - m)
                ng = mw // 128
                tok_mt = ztok.tile([C, MT], tb_dt, tag="tok")
                nc.sync.dma_start(out=tok_mt[:, :mw], in_=tokT[:, m:m + mw])
                coo_mt = zcoo.tile([4, MT], tb_dt, tag="coo")
                nc.sync.dma_start(out=coo_mt[:, :mw], in_=cooT[:, m:m + mw])
                zs = zstage.tile([128, NG, 2 * C], tb_dt, tag="zs")
                for g in range(ng):
                    cc = g * 128
                    ps = zpsum.tile([128, 2 * C], mybir.dt.float32, tag="ps")
                    nc.tensor.matmul(ps[:], lhsT=tok_mt[:, cc:cc + 128],
                                     rhs=w1ab_sb[:], start=True, stop=False)
                    nc.tensor.matmul(ps[:], lhsT=coo_mt[:, cc:cc + 128],
                                     rhs=w1c3_sb[:], start=False, stop=True)
                    # split the PSUM drain across ACT and DVE
                    nc.scalar.copy(out=zs[:, g, 0:C], in_=ps[:, 0:C])
                    nc.vector.tensor_copy(out=zs[:, g, C:2 * C], in_=ps[:, C:2 * C])
                # one batched table write per macro tile: row m+g*128+p <- zs[p, g, :]
                zv = z_dram[m:m + mw, :].rearrange("(g p) c -> p g c", p=128)
                zw = nc.sync.dma_start(out=zv, in_=zs[:, :ng, :])
                zwrites.append(zw.ins)

            # ---------------- Phase G: gather + edge tail ----------------
            # gathers are ordered against z-table writes via explicit per-tile
            # deps (tile_bounds) + sliced in_aps, so phase G overlaps phase Z
            from concourse.tile_rust import add_dep_helper

            # enumerate (bucket, tile) pairs, then emit in ascending order of
            # the z-row bound each tile waits on: the GpSimd engine executes
            # gathers in order, so a late-bound tile must not block ready ones
            tiles = []
            tt = 0
            off = 0
            for b in range(4):
                for t in range(caps[b] // TILE_T):
                    tiles.append((max(tile_bounds[tt]), b, off + t * TILE_T, tt))
                    tt += 1
                off += caps[b]
            tiles.sort()

            qn = 0
            for _bound, b, tile_off, tt in tiles:
                    src_hi, dst_hi = b >= 2, (b % 2) == 1
                    lo1 = HALF if src_hi else 0
                    lo2 = HALF if dst_hi else 0
                    col0 = tile_off // 16
                    cols = TILE_T // 16
                    b1g, b2g = tile_bounds[tt]
                    # slice the source to just the rows this tile touches, so
                    # Tile's DRAM RAW tracking only orders against the z-writes
                    # that actually cover them (lets gathers overlap phase Z)
                    hi1t = min(N_PAD, -(-(b1g + 1) // MT) * MT)
                    hi2t = min(N_PAD, -(-(b2g + 1) // MT) * MT)
                    z1_ap = z_dram[lo1:hi1t, 0:C]
                    z2_ap = z_dram[lo2:hi2t, C:2 * C]
                    g1 = gpool.tile([128, GP, C], tb_dt, tag="g1")
                    gi1 = nc.gpsimd.dma_gather(
                        out_ap=g1[:], in_ap=z1_ap,
                        idxs_ap=sidx_sb[:, col0:col0 + cols],
                        num_idxs=TILE_T, num_idxs_reg=TILE_T,
                        elem_size=C, elem_step=2 * C, queue_num=qn % NQ)
                    add_dep_helper(gi1.ins, zwrites[b1g // MT],
                                   reason="z rows ready for g1")
                    qn += 1
                    g2 = gpool.tile([128, GP, C], tb_dt, tag="g2")
                    gi2 = nc.gpsimd.dma_gather(
                        out_ap=g2[:], in_ap=z2_ap,
                        idxs_ap=didx_sb[:, col0:col0 + cols],
                        num_idxs=TILE_T, num_idxs_reg=TILE_T,
                        elem_size=C, elem_step=2 * C, queue_num=qn % NQ)
                    add_dep_helper(gi2.ins, zwrites[b2g // MT],
                                   reason="z rows ready for g2")
                    qn += 1
                    s = spool.tile([128, GP, C], tb_dt, tag="s")
                    nc.vector.tensor_add(s[:], g1[:], g2[:])
                    r = rpool.tile([128, GP, C], tb_dt, tag="r")
                    nc.scalar.activation(r[:], s[:],
                                         mybir.ActivationFunctionType.Relu)
                    o_pos = opool.tile([128, GP], mybir.dt.float32, tag="op")
                    o_neg = opool.tile([128, GP], mybir.dt.float32, tag="on")
                    if p_pos > 0:
                        nc.vector.reduce_sum(o_pos[:], r[:, :, 0:p_pos],
                                             axis=mybir.AxisListType.X)
                    else:
                        nc.vector.memset(o_pos[:], 0.0)
                    if p_pos < C:
                        nc.vector.reduce_sum(o_neg[:], r[:, :, p_pos:C],
                                             axis=mybir.AxisListType.X)
                    else:
                        nc.vector.memset(o_neg[:], 0.0)
                    nc.vector.tensor_sub(
                        outbuf[:, tt * GP:(tt + 1) * GP], o_pos[:], o_neg[:])
            nc.sync.dma_start(out=outd[:], in_=outbuf[:])

    nc.compile()
    return nc


def _prep_host(tokens, coords, edge_index, w1, b1, w2, b2):
    tokens = np.asarray(tokens, dtype=np.float32)[0]          # [N, C]
    coords = np.asarray(coords, dtype=np.float32)[0]          # [N, 2]
    ei = np.asarray(edge_index).astype(np.int64)              # [2, E]
    w1 = np.asarray(w1, dtype=np.float32)
    b1 = np.asarray(b1, dtype=np.float32)
    w2 = np.asarray(w2, dtype=np.float32)
    b2 = np.asarray(b2, dtype=np.float32)

    w2v = w2[:, 0]
    order = np.argsort(w2v < 0, kind="stable")
    p_pos = int((w2v >= 0).sum())
    scale = np.abs(w2v[order])
    w1p = w1[:, order] * scale[None, :]
    b1p = b1[order] * scale

    W1a, W1b = w1p[:C], w1p[C:2 * C]
    W1cx, W1cy = w1p[2 * C], w1p[2 * C + 1]

    np_tb = ml_dtypes.bfloat16 if TBL_BF16 else np.float32
    tokT_np = np.zeros((C, N_PAD), dtype=np.float32)
    tokT_np[:, :N] = tokens.T
    tokT_np = tokT_np.astype(np_tb)
    cooT_np = np.zeros((4, N_PAD), dtype=np.float32)
    cooT_np[0, :N] = coords[:, 0]
    cooT_np[1, :N] = coords[:, 1]
    cooT_np[2, :] = 1.0
    w1ab_np = np.concatenate([W1a, W1b], axis=1).astype(np_tb)  # [C, 256]
    w1c3_np = np.zeros((4, 2 * C), dtype=np.float32)
    w1c3_np[0] = np.concatenate([-W1cx, W1cx])
    w1c3_np[1] = np.concatenate([-W1cy, W1cy])
    w1c3_np[2] = np.concatenate([np.zeros(C, np.float32), b1p])
    cooT_np = cooT_np.astype(np_tb)
    w1c3_np = w1c3_np.astype(np_tb)

    src, dst = ei[0], ei[1]
    per_core = []
    counts = np.zeros((N_CORES, 4), dtype=np.int64)
    for c in range(N_CORES):
        s_ = src[c * EC:(c + 1) * EC]
        d_ = dst[c * EC:(c + 1) * EC]
        bid = (s_ >= HALF).astype(np.int64) * 2 + (d_ >= HALF).astype(np.int64)
        bound = np.maximum(s_, d_)
        pos = []
        for k in range(4):
            pk = np.nonzero(bid == k)[0]
            pos.append(pk[np.argsort(bound[pk], kind="stable")])
        for k in range(4):
            counts[c, k] = len(pos[k])
        per_core.append((s_, d_, pos))

    caps = tuple(int(-(-counts[:, k].max() // TILE_T) * TILE_T) for k in range(4))
    CAPT = sum(caps)

    in_maps = []
    pos_maps = []
    # per-tile max z-row needed, maximized across cores (SPMD: one program)
    NTT = CAPT // TILE_T
    tb1 = np.zeros(NTT, dtype=np.int64)
    tb2 = np.zeros(NTT, dtype=np.int64)
    for c in range(N_CORES):
        s_, d_, pos = per_core[c]
        sl = np.zeros(CAPT, dtype=np.int16)
        dl = np.zeros(CAPT, dtype=np.int16)
        pm = np.full(CAPT, -1, dtype=np.int64)
        off = 0
        for k in range(4):
            p = pos[k]
            n = len(p)
            sl[off:off + n] = s_[p] - (HALF if k >= 2 else 0)
            dl[off:off + n] = d_[p] - (HALF if (k % 2) == 1 else 0)
            pm[off:off + n] = p
            off += caps[k]
        base1 = np.zeros(CAPT, dtype=np.int64)
        base2 = np.zeros(CAPT, dtype=np.int64)
        o2 = 0
        for k in range(4):
            base1[o2:o2 + caps[k]] = HALF if k >= 2 else 0
            base2[o2:o2 + caps[k]] = HALF if (k % 2) == 1 else 0
            o2 += caps[k]
        r1 = sl.astype(np.int64) + base1
        r2 = dl.astype(np.int64) + base2
        np.maximum(tb1, r1.reshape(NTT, TILE_T).max(1), out=tb1)
        np.maximum(tb2, r2.reshape(NTT, TILE_T).max(1), out=tb2)
        sw = np.tile(np.ascontiguousarray(sl.reshape(-1, 16).T), (8, 1))
        dw = np.tile(np.ascontiguousarray(dl.reshape(-1, 16).T), (8, 1))
        in_maps.append({
            "tokT": tokT_np, "cooT": cooT_np, "w1ab": w1ab_np, "w1c3": w1c3_np,
            "sidx": np.ascontiguousarray(sw), "didx": np.ascontiguousarray(dw),
        })
        pos_maps.append(pm)
    tile_bounds = tuple(zip(tb1.tolist(), tb2.tolist()))
    return caps, p_pos, in_maps, pos_maps, float(b2[0]), tile_bounds


def kernel(tokens, coords, edge_index, w1, b1, w2, b2):
    from concourse.bass_utils import run_bass_kernel_spmd

    caps, p_pos, in_maps, pos_maps, b2v, tile_bounds = _prep_host(
        tokens, coords, edge_index, w1, b1, w2, b2)

    key = (caps, p_pos, tile_bounds)
    if key not in _cache:
        _cache[key] = _build(caps, p_pos, tile_bounds)
    nc = _cache[key]

    last_err = None
    for _attempt in range(3):
        try:
            res = run_bass_kernel_spmd(nc, in_maps, list(range(N_CORES)))
            break
        except Exception as e:  # transient NRT exec-unit errors observed
            last_err = e
            import time as _time
            _time.sleep(20)
    else:
        raise last_err

    CAPT = sum(caps)
    NTT = CAPT // TILE_T
    out = np.empty(E, dtype=np.float32)
    for c in range(N_CORES):
        o = res.results[c]["out"]                   # [128, NTT*GP]
        # element [p, tt*GP + g] is bucket-position tt*TILE_T + g*128 + p
        r = o.reshape(128, NTT, GP).transpose(1, 2, 0).reshape(-1)
        pm = pos_maps[c]
        valid = pm >= 0
        out[c * EC + pm[valid]] = r[valid]
    out += b2v
    return out.reshape(1, E, 1)
